# revision 15
# baseline (speedup 1.0000x reference)
"""Trainium2 Bass kernel for low-rank-QK multi-head attention.

Reference computation (B=4, S=2048, HIDDEN=2048, HEADS=16, R=128):
    Q = q @ wqs_w + wqs_b                    # [B, S, 16]
    K = k @ wks_w + wks_b                    # [B, S, 16]
    V = k @ wvs_w + wvs_b                    # [B, S, 2048]   (v input unused)
    logits = Q @ K^T / sqrt(128)             # [B, S, S]
    score = softmax(logits, -1)
    out = (score @ V) @ wo_w + wo_b          # [B, S, 2048]

Sharding: 8 cores = (batch b, query-half h).  Each core handles the full
key set of its batch and a 1024-row query slice.

Algebraic restructure (all cheap steps on host, big GEMMs on device):
  - softmax's diag(1/Z) commutes past both weight matmuls:
        out = diag(1/Z) exp(QK^T/sqrt(R)) k (wvs_w @ wo_w) + const_row
    so W = wvs_w @ wo_w is folded on host, deleting one S*H*H GEMM.
  - The QK projections are 0.5% of the FLOPs; computed on host, so the
    device uploads shrink to QT/KT (768 KB) + kn (7.5 MB) + W (8 MB) and
    the logits pipeline starts immediately.
  - bias fold: score rows sum to 1 => wvs_b/wo_b contribute the constant
    row wvs_b @ wo_w + wo_b, added on host.
  - fp8 DoubleRow pairs: the last 4 of 16 key tiles run as two e4m3
    DoubleRow matmuls at 2x PE rate.  exp is scaled by s_e (folded into
    the device exp bias as ln s_e) and kn by 1/s_e, so the scales cancel
    inside the matmul and the DR product accumulates into the same PSUM
    group as the bf16 tiles.  rel err 1.85e-2 vs the 2e-2 gate,
    bitwise-reproducible (verified across runs).

Device phases (per core; operands bf16 except the fp8 pairs, PSUM fp32):
  P2: logitsT_j[k, q] = KT_j^T QT; expT_j = exp(logitsT_j/sqrt(R))
      (ScalarE; fp8 tiles via bias=ln s_e), Z-partials on DVE, plus the
      first 4 of phase 4's j-accumulation chains interleaved so the PE
      never idles while ScalarE works through the exponentials.
  P4: ET[hid, q] = sum_j kn_j^T expT_j           (= (exp @ k)^T)
  Z : partial[k_sub, q] = sum_j expT_j (DVE, ping-pong); bf16-cast; 8
      tiny matmuls vs ones => Z in [q-partition, 1] layout; recip = 1/Z
      (DVE); slotted early into P4 so its latency hides.
  P6: out[q, n] = (sum_c ET_c^T W_c[:, n]) * recip[q], W streamed in
      four 512-column blocks, triple-buffered.

DMA choreography: the tile scheduler hoists dependency-free DMAs, so the
bulk kn/W streams are held behind artificial RAW->WAW gates (throwaway
copies reading KTs / late kn chunks) until the small latency-critical
QT/KT uploads have landed; W streams from ScalarE after the exp loop.
"""

import math
import sys

import numpy as np

if "/opt/trn_rl_repo" not in sys.path:
    sys.path.insert(0, "/opt/trn_rl_repo")

import ml_dtypes

BF = ml_dtypes.bfloat16

HIDDEN = 2048
HEADS = 16
R = 128
B = 4
S = 2048

P = 128
SQ = 1024  # queries per core
SK = 2048  # keys per core (full batch)
HC = HIDDEN // P  # 16 hidden-dim chunks
NKT = SK // P  # 16 key tiles
NQT = SQ // P  # 8 query tiles
N512_Q = SQ // 512  # 2
N512_D = HIDDEN // 512  # 4
NCH = 4  # phase-4 chains interleaved into phase 2
NJB = NKT - 4  # bf16 key tiles; the last four go through fp8 DoubleRow
ISQRT_R = 1.0 / math.sqrt(R)

# Module-level knobs for test harness (harness itself only calls kernel()).
TRACE = False
TRACE_KWARGS = {}
LAST_RESULTS = None

_PROG = None


def _emit(tc, nc, mybir, ap, lnse_f, invse_f):
    """Emit the single-core SPMD program body.

    lnse_f/invse_f: ln(s_e) and 1/s_e for the fp8 DoubleRow pairs, baked in
    as compile-time constants (s_e is global across cores; e4m3 relative
    error is scale-free so per-core scaling buys nothing, and the [128,1]
    DMAs these used to be cost ~9us of 4-byte-descriptor trickle that
    stalled the late exp tiles)."""
    from contextlib import ExitStack

    f32 = mybir.dt.float32
    bf16 = mybir.dt.bfloat16
    f8e4 = mybir.dt.float8e4
    Exp = mybir.ActivationFunctionType.Exp
    Add = mybir.AluOpType.add
    DR = mybir.MatmulPerfMode.DoubleRow

    with ExitStack() as ctx:
        # ---- long-lived small tiles -------------------------------------
        small = ctx.enter_context(tc.tile_pool(name="small", bufs=1))
        QTs = small.tile([P, SQ], bf16, name="QTs")  # zero-padded 16->128
        KTs = small.tile([P, SK], bf16, name="KTs")
        onesb = small.tile([P, 1], bf16, name="onesb")
        partA = small.tile([P, SQ], f32, name="partA")  # Z partial ping
        partB = small.tile([P, SQ], f32, name="partB")  # Z partial pong
        partbf = small.tile([P, SQ], bf16, name="partbf")  # bf16 cast for Z
        recip = small.tile([P, NQT], f32, name="recip")
        warm = small.tile([P, 512], bf16, name="warm")  # PE warm-up fodder
        lnse = small.tile([P, 1], f32, name="lnse")  # exp bias, memset const

        expT_pool = tc.alloc_tile_pool(name="expT", bufs=1)
        expT = expT_pool.tile([P, NJB, SQ], bf16, name="expT")
        expT8 = expT_pool.tile([P, 4, SQ], f8e4, name="expT8")
        kn_pool = tc.alloc_tile_pool(name="knp", bufs=1)
        kn_sb = kn_pool.tile([P, NJB, HIDDEN], bf16, name="kn_sb")
        kn8_sb = kn_pool.tile([P, 4, HIDDEN], f8e4, name="kn8_sb")
        ET_pool = tc.alloc_tile_pool(name="ETp", bufs=1, side="right")
        ET = ET_pool.tile([P, HC, SQ], bf16, name="ET")
        wn_pool = tc.alloc_tile_pool(name="wnp", bufs=3, side="right")

        # QT/KT arrive host-padded to 128 rows and are the FIRST DMAs
        # issued, so they own the DMA engines and land in ~2us (the DMA
        # engines round-robin across active queues, so letting the bulk
        # streams start immediately would stretch these small transfers by
        # ~5us and stall the first logits matmul).
        # Split so logits_0 (which reads only KT cols 0:128 / QT cols
        # 0:512) can start as soon as the first slivers land; the tile
        # framework tracks subregion dependencies.
        nc.sync.dma_start(KTs[:, 0:P], ap["KT"][:, 0:P])
        nc.sync.dma_start(QTs[:, 0:512], ap["QT"][:, 0:512])
        nc.sync.dma_start(QTs[:, 512:SQ], ap["QT"][:, 512:SQ])
        nc.sync.dma_start(KTs[:, P:SK], ap["KT"][:, P:SK])
        nc.vector.memset(warm[:], 0.0)
        nc.vector.memset(onesb[:], 1.0)
        nc.vector.memset(lnse[:], lnse_f)
        # Wave 1: the kn chunk the first interleaved chain needs.
        nc.sync.dma_start(kn_sb[:, 0, :], ap["kn"][0:P, :])
        # Gates: these throwaway copies read KTs (so they wait for the
        # QT/KT uploads to land) and write the heads of later DMA targets,
        # making those DMAs queue behind them.  The tile scheduler hoists
        # dependency-free DMAs to the front, so an artificial RAW->WAW
        # chain is the only way to keep the 8 MB kn flood (and even the
        # 4-byte-descriptor lnse/invse transfers, which stall the engines
        # surprisingly long) off the DMA engines while the small
        # latency-critical uploads land.  The garbage the copies write is
        # overwritten by the gated DMAs.
        nc.vector.tensor_copy(kn_sb[:, 1, 0:8], KTs[:, 0:8])
        for j in range(1, NJB):
            nc.sync.dma_start(kn_sb[:, j, :], ap["kn"][j * P:(j + 1) * P, :])
        nc.sync.dma_start(kn8_sb[:], ap["kn8"][:])

        wts = {}

        def load_wn(n, gate=None):
            # W blocks are gated (same trick) on late kn chunks so the
            # 12 MB W stream starts only after kn has landed; W0 is first
            # needed ~90us later.
            wt = wn_pool.tile([P, HC, 512], bf16, name="wn", tag="wn")
            if gate is not None:
                nc.vector.tensor_copy(wt[:, 0, 0:8], kn_sb[:, gate, 0:8])
            nc.scalar.dma_start(wt[:], ap["W"][:, :, n * 512:(n + 1) * 512])
            wts[n] = wt

        # Warm the PE p-state (0.65->2.4 GHz after ~3us busy) while the
        # QT/KT DMAs are in flight; results are discarded.
        with tc.tile_pool(name="ps_w", bufs=1, space="PSUM") as ps_wp:
            pw = ps_wp.tile([P, 512], f32, name="ps_w")
            # Sized so the train ends just before the typical QT/KT sliver
            # landing (~12.5us): the PE is in-order, so the train must not
            # overshoot an early landing by much, but every cycle it covers
            # before the landing is pure idle reclaimed (and keeps the
            # p-state ramped).
            for _ in range(7):
                nc.tensor.matmul(pw[:], warm[:, 0:P], warm[:], start=True,
                                 stop=True)

        # ====== phase 2: logits -> exp -> Z partials, + 4 p4 chains ======
        # PSUM: ps_l 2x2 banks + pch 4 banks = 8 (full).
        parts = [partA, partB]
        with tc.tile_pool(name="ps_l", bufs=2, space="PSUM") as ps_l, \
             tc.tile_pool(name="ps_c", bufs=1, space="PSUM") as ps_cp:
            pch = ps_cp.tile([P, NCH, 512], f32, name="pch")

            def emit_chains(j):
                for ci in range(NCH):
                    ht, n = divmod(ci, N512_Q)
                    nc.tensor.matmul(
                        pch[:, ci, :],
                        kn_sb[:, j, ht * P:(ht + 1) * P],
                        expT[:, j, n * 512:(n + 1) * 512],
                        start=(j == 0), stop=False,
                    )

            def emit_chains_dr(pr):
                # The fp8 DoubleRow pairs close each accumulation: exp is
                # scaled by s_e (folded into the activation bias) and kn8
                # by 1/s_e, so the scales cancel and the product lands in
                # the same PSUM group unscaled at 2x PE rate.  Pair 0 only
                # needs exp tiles 12/13, so it is emitted as soon as they
                # exist to fill the PE while ScalarE finishes exp 14/15.
                for ci in range(NCH):
                    ht, n = divmod(ci, N512_Q)
                    nc.tensor.matmul(
                        pch[:, ci, :],
                        kn8_sb[:, 2 * pr:2 * pr + 2, ht * P:(ht + 1) * P],
                        expT8[:, 2 * pr:2 * pr + 2, n * 512:(n + 1) * 512],
                        start=False, stop=(pr == 1), perf_mode=DR,
                    )

            for j in range(NKT):
                pl = ps_l.tile([P, SQ], f32, name="ps_l", tag="lT")
                for n in range(N512_Q):
                    nc.tensor.matmul(
                        pl[:, n * 512:(n + 1) * 512],
                        KTs[:, j * P:(j + 1) * P],
                        QTs[:, n * 512:(n + 1) * 512],
                        start=True, stop=True,
                    )
                if j == NKT - 1:
                    # DR pair 0 only needs exp tiles 12/13; emitted after
                    # the last logits matmul so it fills the PE while
                    # ScalarE finishes exp 14/15 without delaying them.
                    emit_chains_dr(0)
                if j < NJB:
                    nc.scalar.activation(expT[:, j, :], pl[:], Exp,
                                         scale=ISQRT_R)
                    zsrc, zscale = expT[:, j, :], 1.0
                else:
                    nc.scalar.activation(expT8[:, j - NJB, :], pl[:], Exp,
                                         scale=ISQRT_R, bias=lnse[:, 0:1])
                    zsrc, zscale = expT8[:, j - NJB, :], invse_f
                # Z partial accumulation on DVE (ping-pong buffers)
                if j == 0:
                    nc.vector.tensor_copy(parts[0][:], zsrc)
                else:
                    nc.vector.scalar_tensor_tensor(
                        parts[j % 2][:], zsrc, zscale,
                        parts[(j + 1) % 2][:], mybir.AluOpType.mult, Add,
                    )
                # skew chains by one j so PE never waits on ScalarE
                if 1 <= j <= NJB:
                    emit_chains(j - 1)
            emit_chains_dr(1)
            load_wn(0, gate=9)
            load_wn(1, gate=10)
            load_wn(2, gate=11)
            for ci in range(NCH):
                ht, n = divmod(ci, N512_Q)
                nc.vector.tensor_copy(
                    ET[:, ht, n * 512:(n + 1) * 512], pch[:, ci, :]
                )

        # ====== phase 4 (rest): ET[hid, q] = sum_j kn_j^T expT_j =========
        # The Z reduction (8 tiny bf16 matmuls onto query partitions) is
        # slotted after the first iteration: its DVE cast + semaphore
        # latency hides under the matmul stream, and ps_z has its own PSUM
        # bank so it never waits on ps_e evacuations.  recip is first
        # needed by phase 6.
        pfin = parts[(NKT - 1) % 2]
        # outs/ps_o are co-allocated with ps_e (5 PSUM banks total) so the
        # first phase-6 matmul lands on fresh banks instead of waiting for
        # the final ps_e evacuation to drain (~0.8us bubble otherwise).
        with tc.tile_pool(name="ps_z", bufs=1, space="PSUM") as ps_zp, \
             tc.tile_pool(name="ps_e", bufs=3, space="PSUM") as ps_e, \
             tc.tile_pool(name="outs", bufs=6) as outs, \
             tc.tile_pool(name="ps_o", bufs=3, space="PSUM") as ps_o:
            for ht in range(NCH // N512_Q, HC):
                for n in range(N512_Q):
                    pe = ps_e.tile([P, 512], f32, name="ps_e", tag="e")
                    for j in range(NJB):
                        nc.tensor.matmul(
                            pe[:],
                            kn_sb[:, j, ht * P:(ht + 1) * P],
                            expT[:, j, n * 512:(n + 1) * 512],
                            start=(j == 0), stop=False,
                        )
                    for pr in range(2):
                        nc.tensor.matmul(
                            pe[:],
                            kn8_sb[:, 2 * pr:2 * pr + 2, ht * P:(ht + 1) * P],
                            expT8[:, 2 * pr:2 * pr + 2, n * 512:(n + 1) * 512],
                            start=False, stop=(pr == 1), perf_mode=DR,
                        )
                    nc.vector.tensor_copy(
                        ET[:, ht, n * 512:(n + 1) * 512], pe[:]
                    )
                if ht == NCH // N512_Q:
                    nc.vector.tensor_copy(partbf[:], pfin[:])
                    pz = ps_zp.tile([P, NQT], f32, name="ps_z")
                    for i in range(NQT):
                        nc.tensor.matmul(
                            pz[:, i:i + 1], partbf[:, i * P:(i + 1) * P],
                            onesb[:], start=True, stop=True,
                        )
                    nc.vector.reciprocal(recip[:], pz[:])
        # kn/expT stay allocated through phase 6 (SBUF fits): releasing
        # them here injects a pool-release barrier that cost ~0.8us of PE
        # idle at the phase 4 -> 6 transition.

        # ====== phase 6: out[q, n] = (sum_c ET_c^T W_c) * recip ==========
            for n in range(N512_D):
                wt = wts.pop(n)
                for i in range(NQT):
                    po = ps_o.tile([P, 512], f32, name="ps_o", tag="o")
                    for c in range(HC):
                        nc.tensor.matmul(
                            po[:], ET[:, c, i * P:(i + 1) * P], wt[:, c, :],
                            start=(c == 0), stop=(c == HC - 1),
                        )
                    ot = outs.tile([P, 512], f32, name="ot", tag="ot")
                    nc.vector.tensor_scalar_mul(ot[:], po[:], recip[:, i:i + 1])
                    nc.sync.dma_start(
                        ap["out"][i * P:(i + 1) * P, n * 512:(n + 1) * 512],
                        ot[:],
                    )
                if n + 3 < N512_D:
                    load_wn(n + 3)
        kn_pool.release()
        expT_pool.release()
        wn_pool.release()
        ET_pool.release()


def _build_program(lnse_f, invse_f):
    import concourse.tile as tile
    from concourse import bacc, mybir

    f32 = mybir.dt.float32
    bf16 = mybir.dt.bfloat16

    nc = bacc.Bacc(
        "TRN2", debug=False, num_devices=8, dynamic_dma_scratch_size=512
    )

    f8e4 = mybir.dt.float8e4
    ap = {
        "QT": nc.dram_tensor("QT", (P, SQ), bf16, kind="ExternalInput").ap(),
        "KT": nc.dram_tensor("KT", (P, SK), bf16, kind="ExternalInput").ap(),
        "kn": nc.dram_tensor("kn", (SK, HIDDEN), bf16, kind="ExternalInput").ap(),
        "kn8": nc.dram_tensor("kn8", (P, 4, HIDDEN), f8e4, kind="ExternalInput").ap(),
        "W": nc.dram_tensor("W", (P, HC, HIDDEN), bf16, kind="ExternalInput").ap(),
        "out": nc.dram_tensor("out", (SQ, HIDDEN), f32, kind="ExternalOutput").ap(),
    }

    with tile.TileContext(nc) as tc:
        _emit(tc, nc, mybir, ap, lnse_f, invse_f)

    nc.compile()
    return nc


def _get_program(lnse_f, invse_f):
    global _PROG
    if _PROG is None or _PROG[0] != (lnse_f, invse_f):
        _PROG = ((lnse_f, invse_f), _build_program(lnse_f, invse_f))
    return _PROG[1]


def kernel(q, k, v, wqs_w, wqs_b, wks_w, wks_b, wvs_w, wvs_b, wo_w, wo_b):
    global LAST_RESULTS
    from concourse.bass_utils import run_bass_kernel_spmd

    q = np.asarray(q, dtype=np.float32)
    k = np.asarray(k, dtype=np.float32)
    wqs_w = np.asarray(wqs_w, dtype=np.float32)
    wqs_b = np.asarray(wqs_b, dtype=np.float32)
    wks_w = np.asarray(wks_w, dtype=np.float32)
    wks_b = np.asarray(wks_b, dtype=np.float32)
    wvs_w = np.asarray(wvs_w, dtype=np.float32)
    wvs_b = np.asarray(wvs_b, dtype=np.float32)
    wo_w = np.asarray(wo_w, dtype=np.float32)
    wo_b = np.asarray(wo_b, dtype=np.float32)

    # Host-side cheap steps: QK projections (0.5% of FLOPs), W-fold,
    # constant bias row.
    Q = (q.reshape(-1, HIDDEN) @ wqs_w + wqs_b).reshape(B, S, HEADS)
    K = (k.reshape(-1, HIDDEN) @ wks_w + wks_b).reshape(B, S, HEADS)
    W32 = wvs_w @ wo_w
    bias_row = (wvs_b @ wo_w + wo_b).astype(np.float32)

    # Device layout [P, HC, HIDDEN]: partition p holds W rows c*128+p.
    Wd = np.ascontiguousarray(
        W32.astype(BF).reshape(HC, P, HIDDEN).transpose(1, 0, 2)
    )
    kbf = k.astype(BF)
    kn = [np.ascontiguousarray(kbf[b]) for b in range(B)]
    # Host-padded to 128 rows: the device needs no pad memset, so the QT/KT
    # DMAs have no dependencies and land immediately.
    KT = []
    for b in range(B):
        kt = np.zeros((P, SK), BF)
        kt[0:HEADS, :] = K[b].T.astype(BF)
        KT.append(kt)

    # fp8 DoubleRow pairs: the last 4 key tiles (rows 1536-2047) run at 2x
    # PE rate in e4m3.  exp is scaled by s_e (folded into the device exp
    # bias as ln s_e) and kn by 1/s_e, so the scales cancel inside the
    # matmul.  s_e is GLOBAL (max logit over all cores, from the same
    # bf16-rounded Q/K the device uses) and baked into the program as a
    # constant — e4m3 relative error is scale-free, and this removes the
    # [128,1] lnse/invse DMAs whose 4-byte descriptors trickled out over
    # ~9us and stalled the late exp tiles.
    F8 = ml_dtypes.float8_e4m3fn
    PJ = NJB * P  # 1536
    Kb16 = [K[b].astype(BF).astype(np.float32) for b in range(B)]
    Q16 = Q.astype(BF).astype(np.float32)

    lp_max = max(
        float((Q16[b] @ Kb16[b][PJ:, :].T).max()) for b in range(B)
    ) * ISQRT_R
    s_e = 240.0 * 0.95 / math.exp(lp_max)
    lnse_f = float(np.float32(math.log(s_e)))
    invse_f = float(np.float32(1.0 / s_e))

    nc = _get_program(lnse_f, invse_f)

    kn8s = {}
    for b in range(B):
        kn8s[b] = np.ascontiguousarray(
            np.clip(k[b, PJ:, :] * (1.0 / s_e), -240.0, 240.0)
            .astype(F8).reshape(4, P, HIDDEN).transpose(1, 0, 2)
        )

    in_maps = []
    for core in range(8):
        b, h = divmod(core, 2)
        qt = np.zeros((P, SQ), BF)
        Qc = Q[b, h * SQ:(h + 1) * SQ, :].astype(BF)
        qt[0:HEADS, :] = Qc.T
        in_maps.append({
            "QT": qt,
            "KT": KT[b],
            "kn": kn[b],
            "kn8": kn8s[b],
            "W": Wd,
        })

    res = run_bass_kernel_spmd(
        nc, in_maps, core_ids=list(range(8)), trace=TRACE, **TRACE_KWARGS
    )
    LAST_RESULTS = res

    out = np.empty((B, S, HIDDEN), np.float32)
    for core in range(8):
        b, h = divmod(core, 2)
        out[b, h * SQ:(h + 1) * SQ, :] = res.results[core]["out"] + bias_row
    return out



# revision 22
# speedup vs baseline: 1.0069x; 1.0069x over previous
"""Trainium2 Bass kernel for low-rank-QK multi-head attention.

Reference computation (B=4, S=2048, HIDDEN=2048, HEADS=16, R=128):
    Q = q @ wqs_w + wqs_b                    # [B, S, 16]
    K = k @ wks_w + wks_b                    # [B, S, 16]
    V = k @ wvs_w + wvs_b                    # [B, S, 2048]   (v input unused)
    logits = Q @ K^T / sqrt(128)             # [B, S, S]
    score = softmax(logits, -1)
    out = (score @ V) @ wo_w + wo_b          # [B, S, 2048]

Sharding: 8 cores = (batch b, query-half h).  Each core handles the full
key set of its batch and a 1024-row query slice.

Algebraic restructure (all cheap steps on host, big GEMMs on device):
  - softmax's diag(1/Z) commutes past both weight matmuls:
        out = diag(1/Z) exp(QK^T/sqrt(R)) k (wvs_w @ wo_w) + const_row
    so W = wvs_w @ wo_w is folded on host, deleting one S*H*H GEMM.
  - The QK projections are 0.5% of the FLOPs; computed on host, so the
    device uploads shrink to QT/KT (768 KB) + kn (7.5 MB) + W (8 MB) and
    the logits pipeline starts immediately.
  - bias fold: score rows sum to 1 => wvs_b/wo_b contribute the constant
    row wvs_b @ wo_w + wo_b, added on host.
  - fp8 DoubleRow pairs: the last 4 of 16 key tiles run as two e4m3
    DoubleRow matmuls at 2x PE rate.  exp is scaled by s_e (folded into
    the device exp bias as ln s_e) and kn by 1/s_e, so the scales cancel
    inside the matmul and the DR product accumulates into the same PSUM
    group as the bf16 tiles.  rel err 1.85e-2 vs the 2e-2 gate,
    bitwise-reproducible (verified across runs).

Device phases (per core; operands bf16 except the fp8 pairs, PSUM fp32):
  P2: logitsT_j[k, q] = KT_j^T QT; expT_j = exp(logitsT_j/sqrt(R))
      (ScalarE; fp8 tiles via bias=ln s_e), Z-partials on DVE, plus the
      first 4 of phase 4's j-accumulation chains interleaved so the PE
      never idles while ScalarE works through the exponentials.
  P4: ET[hid, q] = sum_j kn_j^T expT_j           (= (exp @ k)^T)
  Z : partial[k_sub, q] = sum_j expT_j (DVE, ping-pong); bf16-cast; 8
      tiny matmuls vs ones => Z in [q-partition, 1] layout; recip = 1/Z
      (DVE); slotted early into P4 so its latency hides.
  P6: out[q, n] = (sum_c ET_c^T W_c[:, n]) * recip[q], W streamed in
      four 512-column blocks, triple-buffered.

DMA choreography: the tile scheduler hoists dependency-free DMAs, so the
bulk kn/W streams are held behind artificial RAW->WAW gates (throwaway
copies reading KTs / late kn chunks) until the small latency-critical
QT/KT uploads have landed; W streams from ScalarE after the exp loop.
"""

import math
import sys

import numpy as np

if "/opt/trn_rl_repo" not in sys.path:
    sys.path.insert(0, "/opt/trn_rl_repo")

import ml_dtypes

BF = ml_dtypes.bfloat16

HIDDEN = 2048
HEADS = 16
R = 128
B = 4
S = 2048

P = 128
SQ = 1024  # queries per core
SK = 2048  # keys per core (full batch)
HC = HIDDEN // P  # 16 hidden-dim chunks
NKT = SK // P  # 16 key tiles
NQT = SQ // P  # 8 query tiles
N512_Q = SQ // 512  # 2
N512_D = HIDDEN // 512  # 4
NCH = 4  # phase-4 chains interleaved into phase 2
NJB = NKT - 4  # bf16 key tiles; the last four go through fp8 DoubleRow
ISQRT_R = 1.0 / math.sqrt(R)

# Module-level knobs for test harness (harness itself only calls kernel()).
TRACE = False
TRACE_KWARGS = {}
LAST_RESULTS = None

_PROG = None


def _emit(tc, nc, mybir, ap, lnse_f, invse_f):
    """Emit the single-core SPMD program body.

    lnse_f/invse_f: ln(s_e) and 1/s_e for the fp8 DoubleRow pairs, baked in
    as compile-time constants (s_e is global across cores; e4m3 relative
    error is scale-free so per-core scaling buys nothing, and the [128,1]
    DMAs these used to be cost ~9us of 4-byte-descriptor trickle that
    stalled the late exp tiles)."""
    from contextlib import ExitStack

    f32 = mybir.dt.float32
    bf16 = mybir.dt.bfloat16
    f8e4 = mybir.dt.float8e4
    Exp = mybir.ActivationFunctionType.Exp
    Add = mybir.AluOpType.add
    DR = mybir.MatmulPerfMode.DoubleRow

    with ExitStack() as ctx:
        # ---- long-lived small tiles -------------------------------------
        small = ctx.enter_context(tc.tile_pool(name="small", bufs=1))
        QTs = small.tile([P, SQ], bf16, name="QTs")  # Q^T at rows 0:16 & 64:80
        KTs = small.tile([P, SK // 2], bf16, name="KTs")  # pair-packed keys
        onesb = small.tile([P, 1], bf16, name="onesb")
        partA = small.tile([P, SQ], f32, name="partA")  # Z partial ping
        partB = small.tile([P, SQ], f32, name="partB")  # Z partial pong
        partbf = small.tile([P, SQ], bf16, name="partbf")  # bf16 cast for Z
        recip = small.tile([P, NQT], f32, name="recip")
        warm = small.tile([P, 512], bf16, name="warm")  # PE warm-up fodder
        lnse = small.tile([P, 1], f32, name="lnse")  # exp bias, memset const

        expT_pool = tc.alloc_tile_pool(name="expT", bufs=1)
        expT = expT_pool.tile([P, NJB, SQ], bf16, name="expT")
        expT8 = expT_pool.tile([P, 4, SQ], f8e4, name="expT8")
        kn_pool = tc.alloc_tile_pool(name="knp", bufs=1)
        kn_sb = kn_pool.tile([P, NJB, HIDDEN], bf16, name="kn_sb")
        kn8_sb = kn_pool.tile([P, 4, HIDDEN], f8e4, name="kn8_sb")
        ET_pool = tc.alloc_tile_pool(name="ETp", bufs=1, side="right")
        ET = ET_pool.tile([P, HC, SQ], bf16, name="ET")
        wn_pool = tc.alloc_tile_pool(name="wnp", bufs=3, side="right")

        # QT/KT arrive host-padded to 128 rows and are the FIRST DMAs
        # issued, so they own the DMA engines and land in ~2us (the DMA
        # engines round-robin across active queues, so letting the bulk
        # streams start immediately would stretch these small transfers by
        # ~5us and stall the first logits matmul).
        # Split so logits_0 (which reads only KT cols 0:128 / QT cols
        # 0:512) can start as soon as the first slivers land; the tile
        # framework tracks subregion dependencies.
        # KT arrives in per-pair chunks so logits pair b never waits on the
        # bulk stream (each chunk rides its own DMA ring and lands in ~us).
        nc.sync.dma_start(KTs[:, 0:P], ap["KT"][:, 0:P])
        nc.sync.dma_start(QTs[:, 0:512], ap["QT"][:, 0:512])
        nc.sync.dma_start(KTs[:, P:2 * P], ap["KT"][:, P:2 * P])
        nc.sync.dma_start(QTs[:, 512:SQ], ap["QT"][:, 512:SQ])
        nc.sync.dma_start(KTs[:, 2 * P:4 * P], ap["KT"][:, 2 * P:4 * P])
        nc.sync.dma_start(KTs[:, 4 * P:SK // 2], ap["KT"][:, 4 * P:SK // 2])
        nc.vector.memset(warm[:], 0.0)
        nc.vector.memset(onesb[:], 1.0)
        nc.vector.memset(lnse[:], lnse_f)
        # Wave 1: the kn chunk the first interleaved chain needs.
        nc.sync.dma_start(kn_sb[:, 0, :], ap["kn"][0:P, :])
        # Gates: these throwaway copies read KTs (so they wait for the
        # QT/KT uploads to land) and write the heads of later DMA targets,
        # making those DMAs queue behind them.  The tile scheduler hoists
        # dependency-free DMAs to the front, so an artificial RAW->WAW
        # chain is the only way to keep the 8 MB kn flood (and even the
        # 4-byte-descriptor lnse/invse transfers, which stall the engines
        # surprisingly long) off the DMA engines while the small
        # latency-critical uploads land.  The garbage the copies write is
        # overwritten by the gated DMAs.
        nc.vector.tensor_copy(kn_sb[:, 1, 0:8], KTs[:, 0:8])
        for j in range(1, NJB):
            nc.sync.dma_start(kn_sb[:, j, :], ap["kn"][j * P:(j + 1) * P, :])
        nc.sync.dma_start(kn8_sb[:], ap["kn8"][:])

        wts = {}

        def load_wn(n, gate=None):
            # W blocks are gated (same trick) on late kn chunks so the
            # 12 MB W stream starts only after kn has landed; W0 is first
            # needed ~90us later.
            wt = wn_pool.tile([P, HC, 512], bf16, name="wn", tag="wn")
            if gate is not None:
                nc.vector.tensor_copy(wt[:, 0, 0:8], kn_sb[:, gate, 0:8])
            nc.scalar.dma_start(wt[:], ap["W"][:, :, n * 512:(n + 1) * 512])
            wts[n] = wt

        # Warm the PE p-state (0.65->2.4 GHz after ~3us busy) while the
        # QT/KT DMAs are in flight; results are discarded.
        with tc.tile_pool(name="ps_w", bufs=1, space="PSUM") as ps_wp:
            pw = ps_wp.tile([P, 512], f32, name="ps_w")
            # Sized so the train ends just before the typical QT/KT sliver
            # landing (~12.5us): the PE is in-order, so the train must not
            # overshoot an early landing by much, but every cycle it covers
            # before the landing is pure idle reclaimed (and keeps the
            # p-state ramped).
            for _ in range(10):
                nc.tensor.matmul(pw[:], warm[:, 0:P], warm[:], start=True,
                                 stop=True)

        # ====== phase 2: logits -> exp -> Z partials, + 4 p4 chains ======
        # PSUM: ps_l 2x2 banks + pch 4 banks = 8 (full).
        parts = [partA, partB]
        with tc.tile_pool(name="ps_l", bufs=2, space="PSUM") as ps_l, \
             tc.tile_pool(name="ps_c", bufs=1, space="PSUM") as ps_cp:
            pch = ps_cp.tile([P, NCH, 512], f32, name="pch")

            def emit_chains(j):
                for ci in range(NCH):
                    ht, n = divmod(ci, N512_Q)
                    nc.tensor.matmul(
                        pch[:, ci, :],
                        kn_sb[:, j, ht * P:(ht + 1) * P],
                        expT[:, j, n * 512:(n + 1) * 512],
                        start=(j == 0), stop=False,
                    )

            def emit_chains_dr(pr):
                # The fp8 DoubleRow pairs close each accumulation: exp is
                # scaled by s_e (folded into the activation bias) and kn8
                # by 1/s_e, so the scales cancel and the product lands in
                # the same PSUM group unscaled at 2x PE rate.  Pair 0 only
                # needs exp tiles 12/13, so it is emitted as soon as they
                # exist to fill the PE while ScalarE finishes exp 14/15.
                for ci in range(NCH):
                    ht, n = divmod(ci, N512_Q)
                    nc.tensor.matmul(
                        pch[:, ci, :],
                        kn8_sb[:, 2 * pr:2 * pr + 2, ht * P:(ht + 1) * P],
                        expT8[:, 2 * pr:2 * pr + 2, n * 512:(n + 1) * 512],
                        start=False, stop=(pr == 1), perf_mode=DR,
                    )

            def z_acc(j, zsrc, zscale):
                # Z partial accumulation on DVE (ping-pong buffers)
                if j == 0:
                    nc.vector.tensor_copy(parts[0][:], zsrc)
                else:
                    nc.vector.scalar_tensor_tensor(
                        parts[j % 2][:], zsrc, zscale,
                        parts[(j + 1) % 2][:], mybir.AluOpType.mult, Add,
                    )

            # Logits contract over only HEADS=16 rows, so each pass runs TWO
            # key tiles concurrently in 32-row PE row-groups (tile_position
            # (0,0) / (64,0)): KTs packs even tiles at partitions 0:16 and
            # odd tiles at 64:80, QTs carries Q^T at both strips.  A pair's
            # two tiles land in one [P, 2, 512] PSUM tile (2 banks) and ONE
            # 2D-AP exp covers both, halving ScalarE's 293ns per-ACTIVATE
            # overhead.  Halves logits PE time AND ScalarE instruction count.
            NPB = NJB // 2  # 6 bf16 pairs; pairs 6,7 are the fp8 tiles
            for b in range(NKT // 2):
                for n in range(N512_Q):
                    pl = ps_l.tile([P, 2, 512], f32, name="ps_l", tag="lT")
                    nc.tensor.matmul(
                        pl[:, 0, :], KTs[0:32, b * P:(b + 1) * P],
                        QTs[0:32, n * 512:(n + 1) * 512],
                        start=True, stop=True, tile_position=(0, 0),
                    )
                    nc.tensor.matmul(
                        pl[:, 1, :], KTs[64:96, b * P:(b + 1) * P],
                        QTs[64:96, n * 512:(n + 1) * 512],
                        start=True, stop=True, tile_position=(64, 0),
                    )
                    if b < NPB:
                        nc.scalar.activation(
                            expT[:, 2 * b:2 * b + 2, n * 512:(n + 1) * 512],
                            pl[:], Exp, scale=ISQRT_R)
                    else:
                        p8 = b - NPB
                        nc.scalar.activation(
                            expT8[:, 2 * p8:2 * p8 + 2, n * 512:(n + 1) * 512],
                            pl[:], Exp, scale=ISQRT_R, bias=lnse[:, 0:1])
                if b == NKT // 2 - 1:
                    # DR pair 0 only needs exp tiles 12/13; emitted after the
                    # last logits matmuls so it fills the PE while ScalarE
                    # finishes the exps of pair 7 without delaying them.
                    emit_chains_dr(0)
                if b < NPB:
                    z_acc(2 * b, expT[:, 2 * b, :], 1.0)
                    z_acc(2 * b + 1, expT[:, 2 * b + 1, :], 1.0)
                else:
                    p8 = b - NPB
                    z_acc(2 * b, expT8[:, 2 * p8, :], invse_f)
                    z_acc(2 * b + 1, expT8[:, 2 * p8 + 1, :], invse_f)
                # skew chains by one pair so PE never waits on ScalarE
                if 1 <= b <= NPB:
                    emit_chains(2 * (b - 1))
                    emit_chains(2 * b - 1)
            emit_chains_dr(1)
            load_wn(0, gate=9)
            load_wn(1, gate=10)
            load_wn(2, gate=11)
            for ci in range(NCH):
                ht, n = divmod(ci, N512_Q)
                nc.vector.tensor_copy(
                    ET[:, ht, n * 512:(n + 1) * 512], pch[:, ci, :]
                )

        # ====== phase 4 (rest): ET[hid, q] = sum_j kn_j^T expT_j =========
        # The Z reduction (8 tiny bf16 matmuls onto query partitions) is
        # slotted after the first iteration: its DVE cast + semaphore
        # latency hides under the matmul stream, and ps_z has its own PSUM
        # bank so it never waits on ps_e evacuations.  recip is first
        # needed by phase 6.
        pfin = parts[(NKT - 1) % 2]
        # outs/ps_o are co-allocated with ps_e (5 PSUM banks total) so the
        # first phase-6 matmul lands on fresh banks instead of waiting for
        # the final ps_e evacuation to drain (~0.8us bubble otherwise).
        with tc.tile_pool(name="ps_z", bufs=1, space="PSUM") as ps_zp, \
             tc.tile_pool(name="ps_e", bufs=3, space="PSUM") as ps_e, \
             tc.tile_pool(name="outs", bufs=6) as outs, \
             tc.tile_pool(name="ps_o", bufs=3, space="PSUM") as ps_o:
            for ht in range(NCH // N512_Q, HC):
                for n in range(N512_Q):
                    pe = ps_e.tile([P, 512], f32, name="ps_e", tag="e")
                    for j in range(NJB):
                        nc.tensor.matmul(
                            pe[:],
                            kn_sb[:, j, ht * P:(ht + 1) * P],
                            expT[:, j, n * 512:(n + 1) * 512],
                            start=(j == 0), stop=False,
                        )
                    for pr in range(2):
                        nc.tensor.matmul(
                            pe[:],
                            kn8_sb[:, 2 * pr:2 * pr + 2, ht * P:(ht + 1) * P],
                            expT8[:, 2 * pr:2 * pr + 2, n * 512:(n + 1) * 512],
                            start=False, stop=(pr == 1), perf_mode=DR,
                        )
                    nc.vector.tensor_copy(
                        ET[:, ht, n * 512:(n + 1) * 512], pe[:]
                    )
                if ht == NCH // N512_Q:
                    nc.vector.tensor_copy(partbf[:], pfin[:])
                    pz = ps_zp.tile([P, NQT], f32, name="ps_z")
                    for i in range(NQT):
                        nc.tensor.matmul(
                            pz[:, i:i + 1], partbf[:, i * P:(i + 1) * P],
                            onesb[:], start=True, stop=True,
                        )
                    nc.vector.reciprocal(recip[:], pz[:])
        # kn/expT stay allocated through phase 6 (SBUF fits): releasing
        # them here injects a pool-release barrier that cost ~0.8us of PE
        # idle at the phase 4 -> 6 transition.

        # ====== phase 6: out[q, n] = (sum_c ET_c^T W_c) * recip ==========
            for n in range(N512_D):
                wt = wts.pop(n)
                for i in range(NQT):
                    po = ps_o.tile([P, 512], f32, name="ps_o", tag="o")
                    for c in range(HC):
                        nc.tensor.matmul(
                            po[:], ET[:, c, i * P:(i + 1) * P], wt[:, c, :],
                            start=(c == 0), stop=(c == HC - 1),
                        )
                    ot = outs.tile([P, 512], f32, name="ot", tag="ot")
                    nc.vector.tensor_scalar_mul(ot[:], po[:], recip[:, i:i + 1])
                    nc.sync.dma_start(
                        ap["out"][i * P:(i + 1) * P, n * 512:(n + 1) * 512],
                        ot[:],
                    )
                if n + 3 < N512_D:
                    load_wn(n + 3)
        kn_pool.release()
        expT_pool.release()
        wn_pool.release()
        ET_pool.release()


def _build_program(lnse_f, invse_f):
    import concourse.tile as tile
    from concourse import bacc, mybir

    f32 = mybir.dt.float32
    bf16 = mybir.dt.bfloat16

    nc = bacc.Bacc(
        "TRN2", debug=False, num_devices=8, dynamic_dma_scratch_size=512
    )

    f8e4 = mybir.dt.float8e4
    ap = {
        "QT": nc.dram_tensor("QT", (P, SQ), bf16, kind="ExternalInput").ap(),
        "KT": nc.dram_tensor("KT", (P, SK // 2), bf16, kind="ExternalInput").ap(),
        "kn": nc.dram_tensor("kn", (SK, HIDDEN), bf16, kind="ExternalInput").ap(),
        "kn8": nc.dram_tensor("kn8", (P, 4, HIDDEN), f8e4, kind="ExternalInput").ap(),
        "W": nc.dram_tensor("W", (P, HC, HIDDEN), bf16, kind="ExternalInput").ap(),
        "out": nc.dram_tensor("out", (SQ, HIDDEN), f32, kind="ExternalOutput").ap(),
    }

    with tile.TileContext(nc) as tc:
        _emit(tc, nc, mybir, ap, lnse_f, invse_f)

    nc.compile()
    return nc


def _get_program(lnse_f, invse_f):
    global _PROG
    if _PROG is None or _PROG[0] != (lnse_f, invse_f):
        _PROG = ((lnse_f, invse_f), _build_program(lnse_f, invse_f))
    return _PROG[1]


def kernel(q, k, v, wqs_w, wqs_b, wks_w, wks_b, wvs_w, wvs_b, wo_w, wo_b):
    global LAST_RESULTS
    from concourse.bass_utils import run_bass_kernel_spmd

    q = np.asarray(q, dtype=np.float32)
    k = np.asarray(k, dtype=np.float32)
    wqs_w = np.asarray(wqs_w, dtype=np.float32)
    wqs_b = np.asarray(wqs_b, dtype=np.float32)
    wks_w = np.asarray(wks_w, dtype=np.float32)
    wks_b = np.asarray(wks_b, dtype=np.float32)
    wvs_w = np.asarray(wvs_w, dtype=np.float32)
    wvs_b = np.asarray(wvs_b, dtype=np.float32)
    wo_w = np.asarray(wo_w, dtype=np.float32)
    wo_b = np.asarray(wo_b, dtype=np.float32)

    # Host-side cheap steps: QK projections (0.5% of FLOPs), W-fold,
    # constant bias row.
    Q = (q.reshape(-1, HIDDEN) @ wqs_w + wqs_b).reshape(B, S, HEADS)
    K = (k.reshape(-1, HIDDEN) @ wks_w + wks_b).reshape(B, S, HEADS)
    W32 = wvs_w @ wo_w
    bias_row = (wvs_b @ wo_w + wo_b).astype(np.float32)

    # Device layout [P, HC, HIDDEN]: partition p holds W rows c*128+p.
    Wd = np.ascontiguousarray(
        W32.astype(BF).reshape(HC, P, HIDDEN).transpose(1, 0, 2)
    )
    kbf = k.astype(BF)
    kn = [np.ascontiguousarray(kbf[b]) for b in range(B)]
    # Host-padded/packed for the row-group logits: even key tiles at
    # partitions 0:16, odd tiles at 64:80 (pair b = tiles 2b, 2b+1 share
    # column block b).  Zero rows elsewhere make the padded contraction safe.
    KT = []
    for b in range(B):
        kt = np.zeros((P, SK // 2), BF)
        KbT = K[b].T.astype(BF)  # [HEADS, SK]
        for pb in range(SK // (2 * P)):
            kt[0:HEADS, pb * P:(pb + 1) * P] = \
                KbT[:, (2 * pb) * P:(2 * pb + 1) * P]
            kt[64:64 + HEADS, pb * P:(pb + 1) * P] = \
                KbT[:, (2 * pb + 1) * P:(2 * pb + 2) * P]
        KT.append(kt)

    # fp8 DoubleRow pairs: the last 4 key tiles (rows 1536-2047) run at 2x
    # PE rate in e4m3.  exp is scaled by s_e (folded into the device exp
    # bias as ln s_e) and kn by 1/s_e, so the scales cancel inside the
    # matmul.  s_e is GLOBAL (max logit over all cores, from the same
    # bf16-rounded Q/K the device uses) and baked into the program as a
    # constant — e4m3 relative error is scale-free, and this removes the
    # [128,1] lnse/invse DMAs whose 4-byte descriptors trickled out over
    # ~9us and stalled the late exp tiles.
    F8 = ml_dtypes.float8_e4m3fn
    PJ = NJB * P  # 1536
    Kb16 = [K[b].astype(BF).astype(np.float32) for b in range(B)]
    Q16 = Q.astype(BF).astype(np.float32)

    lp_max = max(
        float((Q16[b] @ Kb16[b][PJ:, :].T).max()) for b in range(B)
    ) * ISQRT_R
    s_e = 240.0 * 0.95 / math.exp(lp_max)
    lnse_f = float(np.float32(math.log(s_e)))
    invse_f = float(np.float32(1.0 / s_e))

    nc = _get_program(lnse_f, invse_f)

    kn8s = {}
    for b in range(B):
        kn8s[b] = np.ascontiguousarray(
            np.clip(k[b, PJ:, :] * (1.0 / s_e), -240.0, 240.0)
            .astype(F8).reshape(4, P, HIDDEN).transpose(1, 0, 2)
        )

    in_maps = []
    for core in range(8):
        b, h = divmod(core, 2)
        qt = np.zeros((P, SQ), BF)
        Qc = Q[b, h * SQ:(h + 1) * SQ, :].astype(BF)
        qt[0:HEADS, :] = Qc.T
        qt[64:64 + HEADS, :] = Qc.T
        in_maps.append({
            "QT": qt,
            "KT": KT[b],
            "kn": kn[b],
            "kn8": kn8s[b],
            "W": Wd,
        })

    res = run_bass_kernel_spmd(
        nc, in_maps, core_ids=list(range(8)), trace=TRACE, **TRACE_KWARGS
    )
    LAST_RESULTS = res

    out = np.empty((B, S, HIDDEN), np.float32)
    for core in range(8):
        b, h = divmod(core, 2)
        out[b, h * SQ:(h + 1) * SQ, :] = res.results[core]["out"] + bias_row
    return out

